# revision 5
# baseline (speedup 1.0000x reference)
"""AttMatrixCov loss kernel for 8 Trainium2 NeuronCores.

Math
----
Reference:
    loss = sum_{a, i<j} mean((attc[a,i] outer attc[a,j] - I_C)^2)
         + sum_{a, i<j} mean((atts[a,i]^T atts[a,j] - I_W)^2)

Using sum_{c,d}(x_c y_d - d_cd)^2 = |x|^2 |y|^2 - 2 x.y + C and
|S_i^T S_j|_F^2 = <A_i, A_j> with A_t = S_t S_t^T, every pairwise sum
collapses via sum_{i<j} <u_i, u_j> = 1/2 (|sum_t u_t|^2 - sum_t |u_t|^2):

    loss_c[a] = ( 1/2((sum_t n_t)^2 - sum_t n_t^2)
                - (|sum_t c_t|^2 - sum_t n_t) + P*C ) / C^2,  n_t = |attc[a,t]|^2
    loss_s[a] = ( 1/2(|M_a|_F^2 - sum_t |A_t|_F^2)
                - (|R_a|_F^2 - sum_t |S_t|_F^2) + P*W ) / W^2,
                M_a = sum_t A_t,  R_a = sum_t S_t,  P = 28 pairs.

Sharding: 8 cores = (natt=4) x (ntemp halves=2). Each core loads its
4 atts slices (1 MB), computes A_t = S_t S_t^T on the PE (via PE
transpose, fp32), accumulates M_half / R_half^T and per-partition
square-sums. Host combines the per-core partials (the cross-half terms
|M1+M2|^2, |R1+R2|^2, |v1+v2|^2 need both halves).
"""

import numpy as np

NATT, NTEMP, C = 4, 8, 1024
H, W = 256, 256
TL = NTEMP // 2          # ntemp slices per core
NPAIR = NTEMP * (NTEMP - 1) // 2
P = 128
N_CORES = 8

_nc_cache = None


def _build():
    import concourse.bacc as bacc
    import concourse.bass as bass  # noqa: F401
    import concourse.tile as tile
    from concourse import mybir
    from concourse.masks import make_identity

    f32 = mybir.dt.float32
    nc = bacc.Bacc()
    s_in = nc.dram_tensor("s", [TL, H, W], f32, kind="ExternalInput")
    c_in = nc.dram_tensor("c", [TL, C], f32, kind="ExternalInput")
    m_out = nc.dram_tensor("m_out", [H, H], f32, kind="ExternalOutput")
    r_out = nc.dram_tensor("r_out", [W, H], f32, kind="ExternalOutput")
    st_out = nc.dram_tensor("stats", [P, 20], f32, kind="ExternalOutput")

    with tile.TileContext(nc) as tc:
        with (
            tc.tile_pool(name="const", bufs=1) as const_pool,
            tc.tile_pool(name="sload", bufs=4) as sload,
            tc.tile_pool(name="stp", bufs=4) as stp,
            tc.tile_pool(name="acc", bufs=1) as accp,
            tc.tile_pool(name="scr", bufs=2) as scr,
            tc.tile_pool(name="ps_tr", bufs=2, space="PSUM") as ps_tr,
            tc.tile_pool(name="ps_a", bufs=2, space="PSUM") as ps_a,
        ):
            ident = const_pool.tile([P, P], f32)
            make_identity(nc, ident)

            m_acc = accp.tile([P, 2, H], f32)      # M rows mb*128+p
            r_acc = accp.tile([P, 2, H], f32)      # R^T rows wb*128+p
            stats = accp.tile([P, 20], f32)
            # stats cols: 0:4 |A_t|^2 partials, 4:8 |S_t|^2 partials,
            #             8:12 n_t partials, 12:20 v = sum_t attc[a,t]

            # ---- channel branch ----
            c3 = const_pool.tile([P, TL, 8], f32)  # c3[p,t,f] = attc[t, p*8+f]
            nc.sync.dma_start(out=c3, in_=c_in.rearrange("t (p f) -> p t f", p=P))
            csq = scr.tile([P, TL, 8], f32)
            nc.vector.tensor_mul(csq, c3, c3)
            nc.vector.reduce_sum(
                stats[:, 8:12].rearrange("p (a b) -> p a b", b=1),
                csq,
                axis=mybir.AxisListType.X,
            )
            vtmp = scr.tile([P, 2, 8], f32)
            nc.vector.tensor_add(vtmp, c3[:, 0:2, :], c3[:, 2:4, :])
            nc.vector.tensor_add(stats[:, 12:20], vtmp[:, 0, :], vtmp[:, 1, :])

            # ---- spatial branch ----
            for t in range(TL):
                s_nat = sload.tile([P, 2, W], f32)  # [p, hb, w] = S[hb*128+p, w]
                nc.sync.dma_start(
                    out=s_nat, in_=s_in[t].rearrange("(hb p) w -> p hb w", p=P)
                )

                # |S_t|^2 partials (ACT square + free-dim accumulate)
                s_scr = scr.tile([P, 2, W], f32, tag="actscr")
                nc.scalar.activation(
                    out=s_scr,
                    in_=s_nat,
                    func=mybir.ActivationFunctionType.Square,
                    accum_out=stats[:, 4 + t : 5 + t],
                )

                # PE transpose: pst[p, wb, hb, q] = S[hb*128+q, wb*128+p]
                pst = ps_tr.tile([P, 2, 2, P], f32)
                for wb in range(2):
                    for hb in range(2):
                        nc.tensor.transpose(
                            pst[:, wb, hb, :],
                            s_nat[:, hb, wb * P : (wb + 1) * P],
                            ident,
                        )
                stt = stp.tile([P, 2, H], f32)  # [p, wb, h] = S^T[wb*128+p, h]
                nc.vector.tensor_copy(
                    stt.rearrange("p wb (hb q) -> p wb hb q", hb=2), pst
                )

                # R^T accumulate
                if t == 0:
                    nc.vector.tensor_copy(r_acc, stt)
                else:
                    nc.vector.tensor_add(r_acc, r_acc, stt)

                # A_t = S_t S_t^T : out rows m*128+p, cols h'
                psa = ps_a.tile([P, 2, H], f32)
                for m in range(2):
                    for k in range(2):
                        nc.tensor.matmul(
                            psa[:, m, :],
                            lhsT=stt[:, k, m * P : (m + 1) * P],
                            rhs=stt[:, k, :],
                            start=(k == 0),
                            stop=(k == 1),
                        )

                # |A_t|^2 partials
                a_scr = scr.tile([P, 2, H], f32, tag="actscr")
                nc.scalar.activation(
                    out=a_scr,
                    in_=psa,
                    func=mybir.ActivationFunctionType.Square,
                    accum_out=stats[:, t : t + 1],
                )

                # M accumulate
                if t == 0:
                    nc.vector.tensor_copy(m_acc, psa)
                else:
                    nc.vector.tensor_add(m_acc, m_acc, psa)

            nc.sync.dma_start(
                out=m_out.rearrange("(mb p) h -> p mb h", p=P), in_=m_acc
            )
            nc.sync.dma_start(
                out=r_out.rearrange("(wb p) h -> p wb h", p=P), in_=r_acc
            )
            nc.sync.dma_start(out=st_out[:, :], in_=stats)
    nc.finalize()
    return nc


last_results = None


def _ensure_ntff_hook():
    """Register the axon NTFF profile hook if the image's antenv lacks it.

    Only matters when BASS_TRACE=1; harmless otherwise."""
    import sys
    import types

    try:
        import antenv.axon_hooks  # noqa: F401

        return
    except ImportError:
        pass
    try:
        from trn_agent_boot.trn_boot import _ntff_profile_via_ctypes

        hook = _ntff_profile_via_ctypes("/opt/axon/libaxon_pjrt.so")
    except Exception:
        hook = None
    mod = types.ModuleType("antenv.axon_hooks")
    mod.get_axon_ntff_profile_hook = lambda: hook
    mod.set_axon_ntff_profile_hook = lambda h: None
    sys.modules["antenv.axon_hooks"] = mod


def kernel(attc: np.ndarray, atts: np.ndarray) -> np.ndarray:
    global _nc_cache, last_results
    _ensure_ntff_hook()
    from concourse.bass_utils import run_bass_kernel_spmd

    if _nc_cache is None:
        _nc_cache = _build()
    nc = _nc_cache

    in_maps = []
    for core in range(N_CORES):
        a, hhalf = core // 2, core % 2
        sl = slice(hhalf * TL, (hhalf + 1) * TL)
        in_maps.append(
            {
                "s": np.ascontiguousarray(atts[a, sl]).astype(np.float32),
                "c": np.ascontiguousarray(attc[a, sl]).astype(np.float32),
            }
        )

    res = run_bass_kernel_spmd(nc, in_maps, core_ids=list(range(N_CORES)))
    last_results = res
    outs = res.results

    total = 0.0
    for a in range(NATT):
        o1, o2 = outs[2 * a], outs[2 * a + 1]
        M = o1["m_out"].astype(np.float64) + o2["m_out"].astype(np.float64)
        R = o1["r_out"].astype(np.float64) + o2["r_out"].astype(np.float64)
        st1 = o1["stats"].astype(np.float64)
        st2 = o2["stats"].astype(np.float64)
        sumA = st1[:, 0:4].sum() + st2[:, 0:4].sum()
        sumS = st1[:, 4:8].sum() + st2[:, 4:8].sum()
        n_t = np.concatenate([st1[:, 8:12].sum(0), st2[:, 8:12].sum(0)])
        v = (st1[:, 12:20] + st2[:, 12:20]).reshape(-1)

        loss_c = (
            0.5 * (n_t.sum() ** 2 - (n_t**2).sum())
            - ((v**2).sum() - n_t.sum())
            + NPAIR * C
        ) / (C * C)
        loss_s = (
            0.5 * ((M**2).sum() - sumA)
            - ((R**2).sum() - sumS)
            + NPAIR * W
        ) / (W * W)
        total += loss_c + loss_s

    return np.float32(total)


# revision 7
# speedup vs baseline: 1.0063x; 1.0063x over previous
"""AttMatrixCov loss kernel for 8 Trainium2 NeuronCores.

Math
----
Reference:
    loss = sum_{a, i<j} mean((attc[a,i] outer attc[a,j] - I_C)^2)
         + sum_{a, i<j} mean((atts[a,i]^T atts[a,j] - I_W)^2)

Using sum_{c,d}(x_c y_d - d_cd)^2 = |x|^2 |y|^2 - 2 x.y + C and
|S_i^T S_j|_F^2 = <A_i, A_j> with A_t = S_t S_t^T, every pairwise sum
collapses via sum_{i<j} <u_i, u_j> = 1/2 (|sum_t u_t|^2 - sum_t |u_t|^2):

    loss_c[a] = ( 1/2((sum_t n_t)^2 - sum_t n_t^2)
                - (|sum_t c_t|^2 - sum_t n_t) + P*C ) / C^2,  n_t = |attc[a,t]|^2
    loss_s[a] = ( 1/2(|M_a|_F^2 - sum_t |A_t|_F^2)
                - (|R_a|_F^2 - sum_t |S_t|_F^2) + P*W ) / W^2,
                M_a = sum_t A_t,  R_a = sum_t S_t,  P = 28 pairs.

Sharding: 8 cores = (natt=4) x (ntemp halves=2). Each core loads its
4 atts slices (1 MB), computes A_t = S_t S_t^T on the PE, accumulates
M_half / R_half and per-partition square-sums. Host combines the
per-core partials (the cross-half terms |M1+M2|^2 etc. need both).

Precision: A_t matmuls run in bf16 (operands rounded; f32 PSUM
accumulate) — measured end-to-end rel err ~1e-5. A is symmetric, so
only the upper-triangular blocks (A00 | A01, A11) are computed.
Everything else (channel branch, R, |S|^2) stays exact f32.
"""

import numpy as np

NATT, NTEMP, C = 4, 8, 1024
H, W = 256, 256
TL = NTEMP // 2          # ntemp slices per core
NPAIR = NTEMP * (NTEMP - 1) // 2
P = 128
N_CORES = 8

_nc_cache = None


def _build():
    import concourse.bacc as bacc
    import concourse.tile as tile
    from concourse import mybir
    from concourse.masks import make_identity

    f32 = mybir.dt.float32
    bf16 = mybir.dt.bfloat16
    nc = bacc.Bacc()
    s_in = nc.dram_tensor("s", [TL, H, W], f32, kind="ExternalInput")
    c_in = nc.dram_tensor("c", [TL, C], f32, kind="ExternalInput")
    # m_out: [p, 0:256] = M row-block 0 ([M00 | M01]); [p, 256:384] = M11
    m_out = nc.dram_tensor("m_out", [P, 384], f32, kind="ExternalOutput")
    r_out = nc.dram_tensor("r_out", [H, W], f32, kind="ExternalOutput")
    st_out = nc.dram_tensor("stats", [P, 24], f32, kind="ExternalOutput")

    with tile.TileContext(nc) as tc:
        with (
            tc.tile_pool(name="const", bufs=1) as const_pool,
            tc.tile_pool(name="sload", bufs=4) as sload,
            tc.tile_pool(name="hnat", bufs=4) as hnat_pool,
            tc.tile_pool(name="stp", bufs=4) as stp,
            tc.tile_pool(name="acc", bufs=1) as accp,
            tc.tile_pool(name="scr", bufs=2) as scr,
            tc.tile_pool(name="ps_tr", bufs=2, space="PSUM") as ps_tr,
            tc.tile_pool(name="ps_a", bufs=2, space="PSUM") as ps_a,
        ):
            ident = const_pool.tile([P, P], bf16)
            make_identity(nc, ident)

            m_acc = accp.tile([P, 384], f32)
            r_acc = accp.tile([P, 2, W], f32)   # natural: S rows hb*128+p
            stats = accp.tile([P, 24], f32)
            # stats cols: 0:4  |A row0|^2+|A11|^2 partials per t
            #             4:8  |A01|^2 partials per t
            #             8:12 |S_t|^2 partials per t
            #             12:16 n_t partials, 16:24 v = sum_t attc[a,t]

            # ---- channel branch (exact f32) ----
            c3 = const_pool.tile([P, TL, 8], f32)  # c3[p,t,f] = attc[t, p*8+f]
            nc.sync.dma_start(out=c3, in_=c_in.rearrange("t (p f) -> p t f", p=P))
            csq = scr.tile([P, TL, 8], f32)
            nc.vector.tensor_mul(csq, c3, c3)
            nc.vector.reduce_sum(
                stats[:, 12:16].rearrange("p (a b) -> p a b", b=1),
                csq,
                axis=mybir.AxisListType.X,
            )
            vtmp = scr.tile([P, 2, 8], f32)
            nc.vector.tensor_add(vtmp, c3[:, 0:2, :], c3[:, 2:4, :])
            nc.vector.tensor_add(stats[:, 16:24], vtmp[:, 0, :], vtmp[:, 1, :])

            # ---- spatial branch ----
            for t in range(TL):
                s_nat = sload.tile([P, 2, W], f32)  # [p, hb, w] = S[hb*128+p, w]
                nc.sync.dma_start(
                    out=s_nat, in_=s_in[t].rearrange("(hb p) w -> p hb w", p=P)
                )

                # |S_t|^2 partials (ACT square + free-dim accumulate), exact
                s_scr = scr.tile([P, 2, W], f32, tag="actscr")
                nc.scalar.activation(
                    out=s_scr,
                    in_=s_nat,
                    func=mybir.ActivationFunctionType.Square,
                    accum_out=stats[:, 8 + t : 9 + t],
                )

                # R accumulate in natural orientation, exact f32
                if t == 0:
                    nc.vector.tensor_copy(r_acc, s_nat)
                else:
                    nc.vector.tensor_add(r_acc, r_acc, s_nat)

                # bf16 cast (gpsimd, off the critical engines)
                h_nat = hnat_pool.tile([P, 2, W], bf16)
                nc.gpsimd.tensor_copy(h_nat, s_nat)

                # PE transpose (bf16): pst[p, wb, hb, q] = S[hb*128+q, wb*128+p]
                pst = ps_tr.tile([P, 2, 2, P], bf16)
                for wb in range(2):
                    for hb in range(2):
                        nc.tensor.transpose(
                            pst[:, wb, hb, :],
                            h_nat[:, hb, wb * P : (wb + 1) * P],
                            ident,
                        )
                stt = stp.tile([P, 2, H], bf16)  # [p, wb, h] = S^T[wb*128+p, h]
                nc.vector.tensor_copy(
                    stt.rearrange("p wb (hb q) -> p wb hb q", hb=2), pst
                )

                # A_t upper-tri: psa[:,0:256] = [A00 | A01] (rows p),
                #                psa[:,256:384] = A11 (rows 128+p, cols 128+q)
                psa = ps_a.tile([P, 384], f32)
                for k in range(2):
                    nc.tensor.matmul(
                        psa[:, 0:256],
                        lhsT=stt[:, k, 0:P],
                        rhs=stt[:, k, :],
                        start=(k == 0),
                        stop=(k == 1),
                    )
                for k in range(2):
                    nc.tensor.matmul(
                        psa[:, 256:384],
                        lhsT=stt[:, k, P:H],
                        rhs=stt[:, k, P:H],
                        start=(k == 0),
                        stop=(k == 1),
                    )

                # |A_t|^2 partials
                a_scr = scr.tile([P, 384], f32, tag="ascr")
                nc.scalar.activation(
                    out=a_scr,
                    in_=psa,
                    func=mybir.ActivationFunctionType.Square,
                    accum_out=stats[:, t : t + 1],
                )
                a_scr2 = scr.tile([P, P], f32, tag="ascr2")
                nc.scalar.activation(
                    out=a_scr2,
                    in_=psa[:, P:H],
                    func=mybir.ActivationFunctionType.Square,
                    accum_out=stats[:, 4 + t : 5 + t],
                )

                # M accumulate
                if t == 0:
                    nc.vector.tensor_copy(m_acc, psa)
                else:
                    nc.vector.tensor_add(m_acc, m_acc, psa)

            nc.sync.dma_start(out=m_out[:, :], in_=m_acc)
            nc.sync.dma_start(
                out=r_out.rearrange("(hb p) w -> p hb w", p=P), in_=r_acc
            )
            nc.sync.dma_start(out=st_out[:, :], in_=stats)
    nc.finalize()
    return nc


last_results = None


def _ensure_ntff_hook():
    """Register the axon NTFF profile hook if the image's antenv lacks it.

    Only matters when BASS_TRACE=1; harmless otherwise."""
    import sys
    import types

    try:
        import antenv.axon_hooks  # noqa: F401

        return
    except ImportError:
        pass
    try:
        from trn_agent_boot.trn_boot import _ntff_profile_via_ctypes

        hook = _ntff_profile_via_ctypes("/opt/axon/libaxon_pjrt.so")
    except Exception:
        hook = None
    mod = types.ModuleType("antenv.axon_hooks")
    mod.get_axon_ntff_profile_hook = lambda: hook
    mod.set_axon_ntff_profile_hook = lambda h: None
    sys.modules["antenv.axon_hooks"] = mod


def kernel(attc: np.ndarray, atts: np.ndarray) -> np.ndarray:
    global _nc_cache, last_results
    _ensure_ntff_hook()
    from concourse.bass_utils import run_bass_kernel_spmd

    if _nc_cache is None:
        _nc_cache = _build()
    nc = _nc_cache

    in_maps = []
    for core in range(N_CORES):
        a, hhalf = core // 2, core % 2
        sl = slice(hhalf * TL, (hhalf + 1) * TL)
        in_maps.append(
            {
                "s": np.ascontiguousarray(atts[a, sl]).astype(np.float32),
                "c": np.ascontiguousarray(attc[a, sl]).astype(np.float32),
            }
        )

    res = run_bass_kernel_spmd(nc, in_maps, core_ids=list(range(N_CORES)))
    last_results = res
    outs = res.results

    total = 0.0
    for a in range(NATT):
        o1, o2 = outs[2 * a], outs[2 * a + 1]
        mo = o1["m_out"].astype(np.float64) + o2["m_out"].astype(np.float64)
        # reconstruct full symmetric M from upper-tri packing
        M = np.empty((H, H), np.float64)
        M[0:P, :] = mo[:, 0:256]
        M[P:H, P:H] = mo[:, 256:384]
        M[P:H, 0:P] = mo[:, P:256].T
        R = o1["r_out"].astype(np.float64) + o2["r_out"].astype(np.float64)
        st1 = o1["stats"].astype(np.float64)
        st2 = o2["stats"].astype(np.float64)
        # |A|^2 = (|A00|^2 + |A01|^2 + |A11|^2) + |A01|^2
        sumA = (
            st1[:, 0:4].sum() + st1[:, 4:8].sum()
            + st2[:, 0:4].sum() + st2[:, 4:8].sum()
        )
        sumS = st1[:, 8:12].sum() + st2[:, 8:12].sum()
        n_t = np.concatenate([st1[:, 12:16].sum(0), st2[:, 12:16].sum(0)])
        v = (st1[:, 16:24] + st2[:, 16:24]).reshape(-1)

        loss_c = (
            0.5 * (n_t.sum() ** 2 - (n_t**2).sum())
            - ((v**2).sum() - n_t.sum())
            + NPAIR * C
        ) / (C * C)
        M2 = (M[0:P, :] ** 2).sum() + (M[P:H, P:H] ** 2).sum() + (
            M[0:P, P:H] ** 2
        ).sum()
        loss_s = (
            0.5 * (M2 - sumA)
            - ((R**2).sum() - sumS)
            + NPAIR * W
        ) / (W * W)
        total += loss_c + loss_s

    return np.float32(total)
